# revision 13
# baseline (speedup 1.0000x reference)
"""Grouped SwiGLU expert MLP (MoE) on 8 Trainium2 NeuronCores.

Problem: sorted_x [32768, 512] f32, tokens pre-sorted by expert into 8 equal
contiguous segments of 4096 tokens; per-expert SwiGLU MLP
    h12 = x_e @ w12[e].T          (4096, 2816)
    h   = silu(h12[:, :1408]) * h12[:, 1408:]
    out = h @ w3[e].T             (4096, 512)

Sharding: pure expert parallelism — core e owns expert e's weights and its
4096-token segment (sliced host-side from expert_starts), so no device-side
collectives are needed; the host concatenates the per-core outputs.

Device layout is feature-major throughout ("contraction dim on partitions"),
which makes both GEMMs transpose-free on chip:
    xt   = x_e.T   [512, 4096]  fp16
    w12t = w12.T   [512, 2816]  fp16
    w3t  = w3.T    [1408, 512]  fp16
    outT = out.T   [512, 4096]  f32   (host transposes back)
GEMM1 produces H12^T tiles [128h, Nt] (PSUM), SwiGLU runs on ACT+DVE into
fp16 H^T tiles, GEMM2 consumes them directly. fp16 operands run the PE at
1 cycle/row (vs 4 for f32) — same speed and footprint as bf16 with a 10-bit
mantissa (8x lower rounding error; inputs here are well inside fp16 range).
Accumulation is always f32 in PSUM.
"""

import os

import numpy as np
import ml_dtypes

import concourse.bass as bass
import concourse.mybir as mybir
import concourse.tile as tile
from concourse import bacc
from concourse.bass_utils import run_bass_kernel_spmd

N_CORES = 8
D = 512  # d_model
H = 1408  # hidden
TWOH = 2 * H
TPE = 4096  # tokens per expert
NT = 512  # token block (matmul moving free dim, one PSUM bank in f32)
KD = D // 128  # 4 contraction tiles over d
KH = H // 128  # 11 contraction tiles over h
NB = TPE // NT  # token blocks

F16 = mybir.dt.float16
F32 = mybir.dt.float32
NP_F16 = np.dtype(np.float16)

# Results of a traced run (test harness reads these).
last_exec_time_ns = None
last_trace_path = None


def _build():
    # Bacc (not plain Bass): its compile() pass pipeline legalizes sync
    # waits (>=2 waits per instruction are split into event-sem chains),
    # which this image's walrus requires.
    nc = bacc.Bacc("TRN2", target_bir_lowering=False, debug=False, num_devices=N_CORES)
    xt = nc.dram_tensor("xt", [D, TPE], F16, kind="ExternalInput")
    w12t = nc.dram_tensor("w12t", [D, TWOH], F16, kind="ExternalInput")
    w3t = nc.dram_tensor("w3t", [H, D], F16, kind="ExternalInput")
    outT = nc.dram_tensor("outT", [D, TPE], F32, kind="ExternalOutput")

    # GEMM2 is software-pipelined into the GEMM1/SwiGLU loop with this lag:
    # in iteration hh we issue the GEMM2 matmuls consuming ht[hh - LAG], so
    # the PE never waits on the ACT+DVE SwiGLU chain (~1.3us behind).
    LAG = 2

    with tile.TileContext(nc) as tc:
        with (
            tc.tile_pool(name="weights", bufs=1) as wpool,
            tc.tile_pool(name="xin", bufs=1) as xpool,
            tc.tile_pool(name="ht", bufs=2) as hpool,
            tc.tile_pool(name="swi", bufs=4) as spool,
            tc.tile_pool(name="ot", bufs=4) as opool,
            tc.tile_pool(name="pg", bufs=2, space=bass.MemorySpace.PSUM) as pgate,
            tc.tile_pool(name="pu", bufs=2, space=bass.MemorySpace.PSUM) as pup,
            tc.tile_pool(name="po", bufs=1, space=bass.MemorySpace.PSUM) as pacc,
        ):
            w12s = wpool.tile([128, KD, TWOH], F16)
            w3s = wpool.tile([128, KH, D], F16)
            xs = xpool.tile([128, KD, TPE], F16)

            # Each DMA_DIRECT2D costs ~650ns of sequencer issue time and the
            # per-engine dynamic queue executes transfers serially, so use
            # FEW, COALESCED DMAs (all kd/kh groups in one 3D AP) and spread
            # the streams over three parallel issuers/queues:
            #   SP  (qSPDynamicHW):  w12 in hh-paired chunks, later outputs
            #   ACT (qActDynamicHW): x token blocks (tb=0 first)
            #   Pool (SWDGE):        w3
            # Issue order tracks first-block consumption; deps are shadow-
            # memory precise, so matmuls start as soon as their chunk lands.
            xt_r = xt[:, :].rearrange("(kd p) t -> p kd t", p=128)
            w12_r = w12t[:, :].rearrange("(kd p) h -> p kd h", p=128)
            w3_r = w3t[:, :].rearrange("(kh p) d -> p kh d", p=128)

            def dma_w12(c0, c1):
                nc.sync.dma_start(out=w12s[:, :, c0:c1], in_=w12_r[:, :, c0:c1])

            nc.scalar.dma_start(out=xs[:, :, 0:NT], in_=xt_r[:, :, 0:NT])
            for c0, c1 in [(0, 256), (256, 832), (832, 1408)]:
                dma_w12(c0, c1)
                dma_w12(H + c0, H + c1)
            nc.gpsimd.dma_start(out=w3s[:, 0:6, :], in_=w3_r[:, 0:6, :])
            nc.gpsimd.dma_start(out=w3s[:, 6:KH, :], in_=w3_r[:, 6:KH, :])
            for tb in range(1, NB):
                nc.scalar.dma_start(
                    out=xs[:, :, tb * NT : (tb + 1) * NT],
                    in_=xt_r[:, :, tb * NT : (tb + 1) * NT],
                )

            for tb in range(NB):
                tsl = bass.ts(tb, NT)
                ht = hpool.tile([128, KH, NT], F16)
                acc = [
                    pacc.tile([128, NT], F32, name=f"acc{do}", tag=f"acc{do}")
                    for do in range(KD)
                ]

                def gemm2_step(kh):
                    for do in range(KD):
                        nc.tensor.matmul(
                            acc[do][:],
                            w3s[:, kh, do * 128 : (do + 1) * 128],
                            ht[:, kh, :],
                            start=(kh == 0),
                            stop=(kh == KH - 1),
                        )

                for hh in range(KH):
                    ps_g = pgate.tile([128, NT], F32)
                    ps_u = pup.tile([128, NT], F32)
                    for kd in range(KD):
                        nc.tensor.matmul(
                            ps_g[:],
                            w12s[:, kd, hh * 128 : (hh + 1) * 128],
                            xs[:, kd, tsl],
                            start=(kd == 0),
                            stop=(kd == KD - 1),
                        )
                    for kd in range(KD):
                        nc.tensor.matmul(
                            ps_u[:],
                            w12s[:, kd, H + hh * 128 : H + (hh + 1) * 128],
                            xs[:, kd, tsl],
                            start=(kd == 0),
                            stop=(kd == KD - 1),
                        )
                    sil = spool.tile([128, NT], F32)
                    nc.scalar.activation(
                        sil[:], ps_g[:], mybir.ActivationFunctionType.Silu
                    )
                    nc.vector.tensor_mul(ht[:, hh, :], sil[:], ps_u[:])
                    if hh >= LAG:
                        gemm2_step(hh - LAG)
                for kh in range(KH - LAG, KH):
                    gemm2_step(kh)

                # PSUM->SBUF copies split across ACT and DVE; one coalesced
                # output DMA per block (4 separate ones for the last block so
                # its epilogue pipelines with the remaining copies)
                ot = opool.tile([128, KD, NT], F32)
                for do in range(KD):
                    if do % 2 == 0:
                        nc.scalar.copy(ot[:, do, :], acc[do][:])
                    else:
                        nc.vector.tensor_copy(ot[:, do, :], acc[do][:])
                outT_r = outT[:, :].rearrange("(do p) t -> p do t", p=128)
                if tb < NB - 1:
                    nc.sync.dma_start(out=outT_r[:, :, tsl], in_=ot[:])
                else:
                    for do in range(KD):
                        nc.sync.dma_start(
                            out=outT_r[:, do, tsl], in_=ot[:, do, :]
                        )
    nc.compile()
    return nc


_nc_cache = None


def _get_nc():
    global _nc_cache
    if _nc_cache is None:
        _nc_cache = _build()
    return _nc_cache


def kernel(sorted_x, w12, w3, expert_starts, expert_ends):
    global last_exec_time_ns, last_trace_path
    sorted_x = np.asarray(sorted_x)
    w12 = np.asarray(w12)
    w3 = np.asarray(w3)
    starts = np.asarray(expert_starts).astype(np.int64)
    T = sorted_x.shape[0]

    in_maps = []
    for e in range(N_CORES):
        # jax.lax.dynamic_slice clamps the start index the same way
        s = int(min(max(starts[e], 0), T - TPE))
        xe = sorted_x[s : s + TPE]  # (TPE, D) f32
        in_maps.append(
            {
                "xt": np.ascontiguousarray(xe.T).astype(NP_F16),
                "w12t": np.ascontiguousarray(w12[e].T).astype(NP_F16),
                "w3t": np.ascontiguousarray(w3[e].T).astype(NP_F16),
            }
        )

    trace = bool(os.environ.get("BASS_MOE_TRACE"))
    res = run_bass_kernel_spmd(
        _get_nc(), in_maps, core_ids=list(range(N_CORES)), trace=trace
    )
    if trace:
        last_exec_time_ns = res.exec_time_ns
        iat = res.instructions_and_trace
        last_trace_path = iat[1] if iat else None

    out = np.empty((N_CORES * TPE, D), dtype=np.float32)
    for e in range(N_CORES):
        out[e * TPE : (e + 1) * TPE] = res.results[e]["outT"].T
    return out
